# revision 12
# baseline (speedup 1.0000x reference)
"""Trainium2 Bass kernel for nn_CandidateFinder (retrieval_knn).

Reference semantics: for each query row i (batch b), find the ascending list of
key indices j whose binarized 64-bit vector exactly equals the query's
binarized vector; truncate/pad to 64 with -1 (float32 output [B, L, 64]).

Mapping bits {0,1} -> {-0.5,+0.5}: full 64-bit equality  <=>
    S(i,j) = sum_d qs[i,d]*ks[j,d] == 16      (non-match S <= 15.5)

Device work (8 cores, data-parallel over the 8192 query rows; keys of the
row's batch replicated): a bf16 +-0.5 GEMM [1024,64]@[64,4096] -> S in PSUM,
with per-row match counts reduced out of PSUM by DVE (is_ge + sum) on the
low half of every group and ACT (relu + sum) on the high half. The match
output itself is a constant -1 matrix except for (astronomically rare,
exactly-counted) rows with a match; those are patched on host from the flag
counts, so the result is exact for every input.

The PE stream is the roofline: 64 back-to-back 512-col matmuls at 427ns
(1.2 GHz; the HAM clock gate never releases in this environment). The
program keeps the tensor queue free of all but one wait + one inc per
4-matmul group so the stream runs at the 427ns floor, starts the first
matmul as soon as the first 64KB key chunk + 32KB of queries land, and
splits the last group's reduction across both engines to shorten the tail.
"""

import sys
import types

import numpy as np
import ml_dtypes

import concourse.bacc as bacc
import concourse.mybir as mybir
from concourse.bass_utils import run_bass_kernel_spmd

# If BASS_TRACE is set in the environment but the agent image's antenv lacks
# axon_hooks, run_bass_kernel_spmd would crash on import. Provide a None-hook
# shim so tracing degrades to "skipped" instead. (A real hook installed by a
# test harness beforehand is left untouched.)
try:
    from antenv.axon_hooks import get_axon_ntff_profile_hook  # noqa: F401
except ImportError:
    import antenv

    _hooks_mod = types.ModuleType("antenv.axon_hooks")
    _hooks_mod.get_axon_ntff_profile_hook = lambda: None
    _hooks_mod.set_axon_ntff_profile_hook = lambda h: None
    antenv.axon_hooks = _hooks_mod
    sys.modules["antenv.axon_hooks"] = _hooks_mod

B, L, D = 2, 4096, 64
KMAX = 64
N_CORES = 8
ROWS_PER_CORE = (B * L) // N_CORES  # 1024
QBLKS = ROWS_PER_CORE // 128  # 8 query blocks of 128 rows
NGRP = 16  # (half, qb) groups; half-major order

_CACHE = {}
LAST_RESULTS = None


# The builder runs from an exec'd string with a fixed pseudo-filename so the
# generated BIR (whose debug frames embed source paths) is byte-identical no
# matter where kernel.py lives -- this keeps the on-disk neuron compile cache
# valid across directories/processes.
_BUILDER_SRC = '''
import concourse.bacc as bacc
import concourse.mybir as mybir

B, L, D = 2, 4096, 64
ROWS_PER_CORE = (B * L) // 8
QBLKS = ROWS_PER_CORE // 128
NGRP = 16
JB = 512


def _build_nc():
    # The constructor's all_engine_barrier only guards the const-AP memsets
    # (0.0/1.0 etc.), which this kernel never reads - skip the EVSEM chain
    # it would put at the head of the NEFF.
    import concourse.bass as _bass

    _orig_barrier = _bass.Bass.all_engine_barrier
    _bass.Bass.all_engine_barrier = lambda self, **kw: None
    try:
        nc = bacc.Bacc(
            trn_type="TRN2",
            target_bir_lowering=False,
            disable_frame_to_traceback=True,
        )
    finally:
        _bass.Bass.all_engine_barrier = _orig_barrier
    qsT = nc.dram_tensor(
        "qst", [D, ROWS_PER_CORE], mybir.dt.bfloat16, kind="ExternalInput"
    )
    ksT = nc.dram_tensor("kst", [D, L], mybir.dt.bfloat16, kind="ExternalInput")
    # interleaved per-group flag counts: col 2g = DVE half, col 2g+1 = ACT
    # half, col 32 = the last group's split ACT piece. Groups 0-13 (cols
    # 0:28) are flushed while the matmul stream still runs; only cols 28:33
    # ride the kernel tail.
    flags = nc.dram_tensor(
        "flags", [128, 2 * NGRP + 1], mybir.dt.float32, kind="ExternalOutput"
    )

    from contextlib import ExitStack

    ctx = ExitStack()
    with ctx:
        def sb(name, shape, dt):
            return ctx.enter_context(nc.sbuf_tensor(name, shape, dt))

        def psum(name, shape):
            return ctx.enter_context(
                nc.psum_tensor(name, shape, mybir.dt.float32)
            )

        def sem(name):
            return ctx.enter_context(nc.semaphore(name))

        q_tile = sb("q_tile", [D, ROWS_PER_CORE], mybir.dt.bfloat16)
        k_tile = sb("k_tile", [D, L], mybir.dt.bfloat16)
        fl = sb("fl", [128, 2 * NGRP + 1], mybir.dt.float32)
        tr_dve = sb("tr_dve", [128, 1024], mybir.dt.bfloat16)
        tr_act = sb("tr_act", [128, 1024], mybir.dt.bfloat16)
        act_bias = sb("act_bias", [128, 1], mybir.dt.float32)
        act_warm = sb("act_warm", [128, 1], mybir.dt.float32)
        ps0 = psum("ps0", [128, 2048])
        ps1 = psum("ps1", [128, 2048])
        psb = [ps0, ps1]

        dma_k = sem("dma_k")    # +16 per k chunk (4x512-col, then 2048-col)
        dma_q = sem("dma_q")    # +16 q blocks 0-1, +16 blocks 2-7
        setup = sem("setup")    # act_bias memset done
        red = sem("red")        # each reducer half-group done -> +1
        mm_g = sem("mm_g")      # PE: group g fully written -> >= g+1
        mm_lo15 = sem("mm_lo15")  # PE: last group's banks 0-1 done
        mm_b215 = sem("mm_b215")  # PE: last group's bank 2 done
        dma_out = sem("dma_out")

        # ---- input DMAs, all on the sync queue (a dma_start issued from
        # the scalar queue faults the exec unit on this runtime). DMA cost
        # here is per-DESCRIPTOR (~20ns each; one per partition row), so the
        # inputs go out in exactly three transfers: the first covers all the
        # keys the 8 first-half groups need.
        nc.sync.dma_start(
            out=k_tile[:, 0:1024], in_=ksT[:, 0:1024]
        ).then_inc(dma_k, 16)
        nc.sync.dma_start(
            out=q_tile[:, 0:256], in_=qsT[:, 0:256]
        ).then_inc(dma_q, 16)
        nc.sync.dma_start(
            out=k_tile[:, 1024:2048], in_=ksT[:, 1024:2048]
        ).then_inc(dma_k, 16)
        nc.sync.wait_ge(dma_k, 32)
        nc.sync.dma_start(
            out=q_tile[:, 256:1024], in_=qsT[:, 256:1024]
        ).then_inc(dma_q, 16)
        nc.sync.dma_start(
            out=k_tile[:, 2048:4096], in_=ksT[:, 2048:4096]
        ).then_inc(dma_k, 16)

        # groups 0-13 flag columns flush under the matmul stream; the tail
        # only carries the last 5 columns. Gate at 31: red is incremented by
        # both reducers in nondeterministic interleave, and with DVE capped
        # at 17 incs and ACT at 16, red >= 31 is the smallest count that
        # PROVES both engines are past group 13 (DVE >= 14, ACT >= 15).
        nc.sync.wait_ge(red, 31)
        nc.sync.dma_start(out=flags[:, 0:28], in_=fl[:, 0:28]).then_inc(dma_out, 16)
        _ = dma_out  # flushed by the walrus epilogue's per-engine DRAIN

        # ---- vector: bias memset, then one 1024-col scan per group ----
        nc.vector.memset(act_bias[:], -15.5).then_inc(setup, 1)
        for g in range(NGRP):
            ps = psb[g % 2]
            if g == NGRP - 1:
                nc.vector.wait_ge(mm_lo15, 1)
            else:
                nc.vector.wait_ge(mm_g, g + 1)
            nc.vector.tensor_scalar(
                out=tr_dve[:],
                in0=ps[:, 0:1024],
                scalar1=15.75,
                scalar2=None,
                op0=mybir.AluOpType.is_ge,
                op1=mybir.AluOpType.add,
                accum_out=fl[:, 2 * g:2 * g + 1],
            ).then_inc(red, 1)


        # ---- scalar (ACT): table-preload during the DMA wait, then one
        # 1024-col relu scan per group (split in two for the last group) ----
        nc.scalar.wait_ge(setup, 1)
        nc.scalar.activation(
            out=act_warm[:], in_=act_bias[:],
            func=mybir.ActivationFunctionType.Relu,
            bias=act_bias[:], scale=1.0,
        )
        for g in range(NGRP - 1):
            ps = psb[g % 2]
            nc.scalar.wait_ge(mm_g, g + 1)
            nc.scalar.activation(
                out=tr_act[:],
                in_=ps[:, 1024:2048],
                func=mybir.ActivationFunctionType.Relu,
                bias=act_bias[:],
                scale=1.0,
                accum_out=fl[:, 2 * g + 1:2 * g + 2],
            ).then_inc(red, 1)
        ps = psb[(NGRP - 1) % 2]
        nc.scalar.wait_ge(mm_b215, 1)
        nc.scalar.activation(
            out=tr_act[:, 0:512], in_=ps[:, 1024:1536],
            func=mybir.ActivationFunctionType.Relu,
            bias=act_bias[:], scale=1.0,
            accum_out=fl[:, 2 * NGRP - 1:2 * NGRP],
        ).then_inc(red, 1)
        nc.scalar.wait_ge(mm_g, NGRP)
        nc.scalar.activation(
            out=tr_act[:, 0:512], in_=ps[:, 1536:2048],
            func=mybir.ActivationFunctionType.Relu,
            bias=act_bias[:], scale=1.0,
            accum_out=fl[:, 2 * NGRP:2 * NGRP + 1],
        ).then_inc(red, 1)
        # final flag columns go out on the ACT ring in program order: no
        # cross-engine semaphore hop on the kernel's last chain.
        nc.scalar.dma_start(
            out=flags[:, 28:33], in_=fl[:, 28:33]
        ).then_inc(dma_out, 16)

        # ---- tensor: the matmul stream; group g = (half, qb) half-major so
        # the second half of the keys is needed only ~14us in ----
        for g in range(NGRP):
            qb = g % QBLKS
            half = g // QBLKS
            ps = psb[g % 2]
            lhsT = q_tile[:, qb * 128:(qb + 1) * 128]
            if g == 0:
                nc.tensor.wait_ge(dma_q, 16)
            if g == 2:
                nc.tensor.wait_ge(dma_q, 32)
            if g == 8:
                nc.tensor.wait_ge(dma_k, 48)
            if g >= 2:
                nc.tensor.wait_ge(red, 2 * (g - 1))
            for bk in range(4):
                if g == 0 and bk in (0, 2):
                    nc.tensor.wait_ge(dma_k, 16 * (bk // 2 + 1))
                mm = nc.tensor.matmul(
                    ps[:, bk * JB:(bk + 1) * JB],
                    lhsT,
                    k_tile[:, half * 2048 + bk * JB:half * 2048 + (bk + 1) * JB],
                    start=True,
                    stop=True,
                )
                if g == NGRP - 1:
                    if bk == 1:
                        mm.then_inc(mm_lo15, 1)
                    elif bk == 2:
                        mm.then_inc(mm_b215, 1)
                    elif bk == 3:
                        mm.then_inc(mm_g, 1)
                elif bk == 3:
                    mm.then_inc(mm_g, 1)

    nc.finalize()
    return nc
'''

_builder_mod = types.ModuleType("cf_builder")
exec(compile(_BUILDER_SRC, "<cf_builder2>", "exec"), _builder_mod.__dict__)
_build_nc = _builder_mod._build_nc


def _get_nc():
    if "nc" not in _CACHE:
        _CACHE["nc"] = _build_nc()
    return _CACHE["nc"]


def _exact_row(q_bits_row, k_bits):
    """Exact reference semantics for one query row given binarized keys."""
    eq = (k_bits == q_bits_row[None, :]).all(axis=1)
    idx = np.nonzero(eq)[0][:KMAX]
    row = np.full(KMAX, -1.0, dtype=np.float32)
    row[: idx.size] = idx.astype(np.float32)
    return row


def kernel(query_up, key_up, head_idx=0):
    global LAST_RESULTS
    q = np.asarray(query_up, dtype=np.float32)  # [B, L, D]
    k = np.asarray(key_up, dtype=np.float32)
    assert q.shape == (B, L, D) and k.shape == (B, L, D)

    # Host prep: binarize to +-0.5 bf16 and transpose to [D, L] per batch so
    # the contraction dim lands on SBUF partitions with no on-device transpose.
    qs = np.where(q > 0, np.float32(0.5), np.float32(-0.5))
    ks = np.where(k > 0, np.float32(0.5), np.float32(-0.5))
    qsT = np.ascontiguousarray(qs.transpose(0, 2, 1)).astype(ml_dtypes.bfloat16)
    ksT = np.ascontiguousarray(ks.transpose(0, 2, 1)).astype(ml_dtypes.bfloat16)

    in_maps = []
    for c in range(N_CORES):
        b = c // (N_CORES // B)
        s = (c % (N_CORES // B)) * ROWS_PER_CORE
        in_maps.append(
            {
                "qst": np.ascontiguousarray(qsT[b][:, s : s + ROWS_PER_CORE]),
                "kst": ksT[b],
            }
        )

    nc = _get_nc()
    res = run_bass_kernel_spmd(nc, in_maps, core_ids=list(range(N_CORES)))
    LAST_RESULTS = res

    out = np.full((B, L, KMAX), -1.0, dtype=np.float32)
    for c in range(N_CORES):
        b = c // (N_CORES // B)
        s = (c % (N_CORES // B)) * ROWS_PER_CORE

        # cols 2g/2g+1 of the flag output cover local rows (g % QBLKS)*128
        # + p; any count > 0.1 => that row has at least one match somewhere.
        fa = res.results[c]["flags"]
        fl = fa[:, 0:2 * NGRP:2] + fa[:, 1:2 * NGRP:2]
        fl[:, NGRP - 1] += fa[:, 2 * NGRP]  # last group's split ACT piece
        ps_, gs = np.nonzero(fl > 0.1)
        if ps_.size:
            k_bits = k[b] > 0
            q_bits = q[b] > 0
            for p, g in zip(ps_, gs):
                i = s + (g % QBLKS) * 128 + p
                out[b, i] = _exact_row(q_bits[i], k_bits)

    return out
